# revision 13
# baseline (speedup 1.0000x reference)
"""AxialMSAEncoderBlock on 8 trn2 NeuronCores — v2.

Strategy (same sharding as v1, restructured for speed):
  Phase 1 (row shard): each core owns 8 of 64 rows (2048 tokens).
    LN1 (scale/bias folded into q/k/v weights on host) -> q,k projections
    interleaved per head-pair with tied row-attention partial scores ->
    bf16 AllReduce split in two (overlaps v projection) -> max-free
    softmax -> context -> out proj + residual (x streamed from DRAM).
  Reshuffle on DVE into per-destination contiguous blocks; bf16 AllToAll
    split in two e-chunk halves; recv side: contiguous DMA + DVE permute
    into SBUF-resident x2 (no DRAM round trip).
  Phase 2 (col shard, 32 cols x 64 rows): LN2 + col-attention with
    pre-zeroed block-diagonal p^T tiles, out proj + residual into
    SBUF-resident fp32 x2p; LN3 + FFN direct from SBUF.

Layout: activations feature-major (E on partitions, tokens on free dim).
All matmuls bf16 with fp32 PSUM accumulation. LN stats via ones-vector
matmuls on PE; broadcasts via K=1 matmuls. No softmax max-subtraction
(|logit| <= ~6 for this distribution; exp is safe in fp32/bf16).
Phase-1 token order: t = r_local*256 + c.  Phase-2: t' = c_local*64 + rg.
"""
import os

os.environ.setdefault("JAX_COMPILATION_CACHE_DIR", "/tmp/jax_cache")

import numpy as np
import ml_dtypes

import concourse.bass as bass
import concourse.mybir as mybir
import concourse.tile as tile
from concourse.bass_utils import run_bass_kernel_spmd
from concourse.masks import make_identity

F32 = mybir.dt.float32
BF16 = mybir.dt.bfloat16
AF = mybir.ActivationFunctionType
ALU = mybir.AluOpType

NCORES = 8
R, C, E, H, D, F = 64, 256, 768, 12, 64, 3072
RL = R // NCORES          # 8 local rows   (phase 1)
CL = C // NCORES          # 32 local cols  (phase 2)
T = 2048                  # local tokens in both phases
EC = E // 128             # 6 e-chunks
FC = F // 128             # 24 f-chunks
NS = T // 512             # 4 moving splits of 512
S1 = (D ** -0.5) / (R ** 0.5)   # row-attn scale (folded into exp)
S2 = D ** -0.5                  # col-attn scale
EPS = 1e-6

_CACHE = {}


def _waitsplit(nc, max_waits=1):
    """walrus accepts only one sync-wait per instruction; split the excess
    onto same-engine NoOps placed immediately before."""
    ctr = 0
    for f in nc.m.functions:
        for bb in f.blocks:
            insts = bb.instructions
            if not any(
                i.sync_info is not None and i.sync_info.on_wait
                and len(i.sync_info.on_wait) > max_waits for i in insts
            ):
                continue
            out = []
            for inst in insts:
                si = inst.sync_info
                waits = list(si.on_wait) if (si is not None and si.on_wait) else []
                if len(waits) > max_waits:
                    extra, keep = waits[:-max_waits], waits[-max_waits:]
                    for w in extra:
                        ctr += 1
                        nop = mybir.InstNoOp(
                            name=f"I-ws-{ctr}", engine=inst.engine, ins=[], outs=[])
                        nop.sync_info = mybir.SyncInfo(on_wait=[w], on_update=[])
                        out.append(nop)
                    inst.sync_info = mybir.SyncInfo(
                        on_wait=keep, on_update=list(si.on_update or []))
                out.append(inst)
            bb.instructions = out


def build_program(debug=False, split=True, stage=3):
    nc = bass.Bass("TRN2", target_bir_lowering=False, debug=False,
                   num_devices=NCORES)

    x_in = nc.declare_dram_parameter("x_fm", [E, T], F32, isOutput=False)
    wnames = ["rq_w", "rk_w", "rv_w", "ro_w", "cq_w", "ck_w", "cv_w", "co_w"]
    wd = {n: nc.declare_dram_parameter(n, [E, E], BF16, isOutput=False)
          for n in wnames}
    wd["f1_w"] = nc.declare_dram_parameter("f1_w", [E, F], BF16, isOutput=False)
    wd["f2_w"] = nc.declare_dram_parameter("f2_w", [F, E], BF16, isOutput=False)
    bnames = ["rq_b", "rk_b", "rv_b", "ro_b", "cq_b", "ck_b", "cv_b", "co_b",
              "f2_b"]
    bd = {n: nc.declare_dram_parameter(n, [E], F32, isOutput=False)
          for n in bnames}
    bd["f1_b"] = nc.declare_dram_parameter("f1_b", [F], F32, isOutput=False)
    y_out = nc.declare_dram_parameter("y", [E, T], F32, isOutput=True)

    with tile.TileContext(nc) as tc:
        _build_body(nc, tc, x_in, wd, bd, y_out, stage)
    if split:
        _waitsplit(nc)
    return nc


def _build_body(nc, tc, x_in, wd, bd, y_out, stage=3):
    from contextlib import ExitStack
    est = ExitStack()
    with est:
        gp = est.enter_context(tc.tile_pool(name="gp", bufs=1))
        dr = est.enter_context(tc.tile_pool(name="dr", bufs=1, space="DRAM"))

        # ---- constants ----
        ident = gp.tile([128, 128], BF16, name="ident")
        make_identity(nc, ident)
        ones_col = gp.tile([128, 1], BF16, name="ones_col")
        nc.vector.memset(ones_col, 1.0)
        ones_row = gp.tile([1, 128], F32, name="ones_row")
        nc.vector.memset(ones_row, 1.0)
        eps_t = gp.tile([1, 1], F32, name="eps_t")
        nc.vector.memset(eps_t, EPS)

        def load_bias_chunks(name, nch=EC):
            """one strided DMA per bias tensor: DRAM [nch*128] ->
            SBUF [128, nch] with chunk m in column m."""
            t = gp.tile([128, nch], F32, name=f"b_{name}")
            full = bd[name][0:nch * 128]
            in_ap = bass.AP(
                tensor=full.tensor, offset=full.offset,
                ap=[[1, 128], [128, nch]])
            nc.sync.dma_start(out=t, in_=in_ap)
            return [t[:, m:m + 1] for m in range(nch)]

        bias = {n: load_bias_chunks(n) for n in
                ["rq_b", "rk_b", "rv_b", "ro_b", "cq_b", "ck_b", "cv_b",
                 "co_b", "f2_b"]}
        bias["f1_b"] = load_bias_chunks("f1_b", FC)

        # x2p: phase-2 residual stream, lives across p2a -> p2b pools.
        x2p = [gp.tile([128, T], BF16, name=f"x2p{m}") for m in range(EC)]

        # DRAM buffers (collective payloads only)
        aw_send = dr.tile([H, C, C], BF16, name="aw_send")
        aw_recv0 = dr.tile([6, C, C], BF16, name="aw_recv0",
                           addr_space="Shared")
        aw_recv1 = dr.tile([6, C, C], BF16, name="aw_recv1",
                           addr_space="Shared")
        # split layout: [half, dest, e-in-half, c*r] so each half is a
        # contiguous AllToAll payload
        a2a_send = dr.tile([2, NCORES, E // 2, CL * RL], BF16,
                           name="a2a_send")
        a2a_recv = dr.tile([2, NCORES, E // 2, CL * RL], BF16,
                           name="a2a_recv")

        # =============== helpers ===============
        def layernorm(getx, pool, htag, xbf16):
            """LN over feature dim (partition reduce by ones-matmul).
            getx(k, s) -> AP [128, 512] (fp32 unless xbf16).
            Returns 6 bf16 (128, T) normalized tiles (no scale/bias: folded
            into downstream weights on host)."""
            def row(nm):
                return pool.tile([1, 512], F32, name=nm, tag="row", bufs=8)
            ht = [pool.tile([128, T], BF16, name=f"h{htag}{k}",
                            tag="h", bufs=EC) for k in range(EC)]
            with tc.tile_pool(name="lnps", bufs=1, space="PSUM") as lnps:
                s_ps = [lnps.tile([1, 512], F32, name=f"sps{s}", tag="sps",
                                  bufs=NS) for s in range(NS)]
                q_ps = [lnps.tile([1, 512], F32, name=f"qps{s}", tag="qps",
                                  bufs=NS) for s in range(NS)]
                for k in range(EC):
                    for s in range(NS):
                        xsl = getx(k, s)
                        if xbf16:
                            xb = xsl
                        else:
                            xbt = pool.tile([128, 512], BF16, name="xb",
                                            tag="xbf", bufs=2)
                            nc.vector.tensor_copy(out=xbt, in_=xsl)
                            xb = xbt
                        xq = pool.tile([128, 512], BF16, name="xq",
                                       tag="xsq", bufs=2)
                        nc.scalar.activation(xq, xsl, AF.Square)
                        nc.tensor.matmul(s_ps[s], ones_col, xb,
                                         start=(k == 0), stop=(k == EC - 1))
                        nc.tensor.matmul(q_ps[s], ones_col, xq,
                                         start=(k == 0), stop=(k == EC - 1))
                for s in range(NS):
                    sl = slice(s * 512, (s + 1) * 512)
                    mean = row("mean")
                    nc.vector.tensor_scalar_mul(mean, s_ps[s], 1.0 / E)
                    q2 = row("q2")
                    nc.vector.tensor_scalar_mul(q2, q_ps[s], 1.0 / E)
                    msq = row("msq")
                    nc.vector.tensor_mul(msq, mean, mean)
                    nc.vector.tensor_sub(q2, q2, msq)       # q2 <- var
                    sd = row("sd")
                    nc.scalar.activation(sd, q2, AF.Sqrt, bias=eps_t)
                    istd = row("istd")
                    nc.vector.reciprocal(istd, sd)
                    bp = row("bp")
                    nc.vector.tensor_mul(bp, mean, istd)
                    ib_ps = lnps.tile([128, 512], F32, name="ibps",
                                      tag="sps", bufs=NS)
                    nc.tensor.matmul(ib_ps, ones_row, istd)
                    istd_b = pool.tile([128, 512], F32, name="istdb",
                                       tag="bc1", bufs=1)
                    nc.vector.tensor_copy(out=istd_b, in_=ib_ps)
                    bp_ps = lnps.tile([128, 512], F32, name="bpps",
                                      tag="qps", bufs=NS)
                    nc.tensor.matmul(bp_ps, ones_row, bp)
                    bp_b = pool.tile([128, 512], F32, name="bpb",
                                     tag="bc2", bufs=1)
                    nc.vector.tensor_copy(out=bp_b, in_=bp_ps)
                    for k in range(EC):
                        t1 = pool.tile([128, 512], F32, name="lnt",
                                       tag="lntmp", bufs=2)
                        nc.vector.tensor_mul(t1, getx(k, s), istd_b)
                        nc.vector.tensor_sub(ht[k][:, sl], t1, bp_b)
            return ht

        def load_w(pool, w_dram, rows, name, tag, bufs):
            ts_ = []
            for k in range(rows // 128):
                t = pool.tile([128, w_dram.shape[1]], BF16, name=f"{name}{k}",
                              tag=tag, bufs=bufs)
                nc.sync.dma_start(out=t, in_=w_dram[k * 128:(k + 1) * 128, :])
                ts_.append(t)
            return ts_

        def project_chunk(ht, w_tiles, b_chunk, m, pool, otag, oname, pps,
                          obufs=2):
            """one output feature chunk m of a feature-major projection."""
            o = pool.tile([128, T], BF16, name=f"{oname}{m}", tag=otag,
                          bufs=obufs)
            for s in range(NS):
                ps = pps.tile([128, 512], F32, name=f"pp{m}_{s}", tag="pp",
                              bufs=3)
                for k in range(EC):
                    nc.tensor.matmul(
                        ps, w_tiles[k][:, m * 128:(m + 1) * 128],
                        ht[k][:, s * 512:(s + 1) * 512],
                        start=(k == 0), stop=(k == EC - 1))
                nc.scalar.activation(
                    o[:, s * 512:(s + 1) * 512], ps, AF.Identity,
                    bias=b_chunk)
            return o

        def project_tm(ht, w_tiles, pool, vtag, vname):
            """token-major projection (for v): 16 tiles [128, E]."""
            out = []
            with tc.tile_pool(name="vps", bufs=1, space="PSUM") as vps:
                for tch in range(T // 128):
                    v = pool.tile([128, E], BF16, name=f"{vname}{tch}",
                                  tag=vtag, bufs=T // 128)
                    for s, (c0, cn) in enumerate([(0, 512), (512, 256)]):
                        ps = vps.tile([128, 512], F32, name=f"vp{tch}_{s}",
                                      tag="vp", bufs=4)
                        for k in range(EC):
                            nc.tensor.matmul(
                                ps[:, :cn],
                                ht[k][:, tch * 128:(tch + 1) * 128],
                                w_tiles[k][:, c0:c0 + cn],
                                start=(k == 0), stop=(k == EC - 1))
                        nc.scalar.activation(v[:, c0:c0 + cn], ps[:, :cn],
                                             AF.Identity)
                    out.append(v)
            return out

        # ================= PHASE 1 (row shard) =================
        with tc.tile_pool(name="p1", bufs=1) as p1:
            def fetch1(k, s):
                t = p1.tile([128, 512], F32, name="xsl", tag="xsl", bufs=4)
                nc.sync.dma_start(
                    out=t, in_=x_in[k * 128:(k + 1) * 128,
                                    s * 512:(s + 1) * 512])
                return t

            h1 = layernorm(fetch1, p1, "1", xbf16=False)
            rq = load_w(p1, wd["rq_w"], E, "rq", "w1", 12)
            rk = load_w(p1, wd["rk_w"], E, "rk", "w1", 12)

            # ---- q/k proj + row-attn scores interleaved per head-pair ----
            with tc.tile_pool(name="sps1", bufs=1, space="PSUM") as sps1, \
                 tc.tile_pool(name="pps1", bufs=1, space="PSUM") as pps1:
                for hc in range(EC):
                    qt = project_chunk(h1, rq, bias["rq_b"][hc], hc, p1,
                                       "q1", "q", pps1)
                    kt = project_chunk(h1, rk, bias["rk_b"][hc], hc, p1,
                                       "k1", "k", pps1)
                    for hb in range(2):
                        h = hc * 2 + hb
                        for ic in range(2):
                            ps = sps1.tile([128, 256], F32, name=f"aw{h}_{ic}",
                                           tag="aw", bufs=4)
                            for r in range(RL):
                                nc.tensor.matmul(
                                    ps,
                                    qt[hb * 64:(hb + 1) * 64,
                                       r * 256 + ic * 128:
                                       r * 256 + (ic + 1) * 128],
                                    kt[hb * 64:(hb + 1) * 64,
                                       r * 256:(r + 1) * 256],
                                    start=(r == 0), stop=(r == RL - 1))
                            sb = p1.tile([128, 256], BF16, name="awsb",
                                         tag="awsb", bufs=4)
                            nc.vector.tensor_copy(out=sb, in_=ps)
                            nc.sync.dma_start(
                                out=aw_send[h, ic * 128:(ic + 1) * 128, :],
                                in_=sb)
                    if hc == 2 and stage != 0:
                        nc.gpsimd.collective_compute(
                            "AllReduce", ALU.add,
                            replica_groups=[list(range(NCORES))],
                            ins=[aw_send[0:6, :, :]],
                            outs=[aw_recv0[:, :, :]])
            if stage == 0:
                def aw_src(h):
                    return aw_send[h]      # timing-only: skip AllReduce
            else:
                def aw_src(h):
                    return (aw_recv0[h] if h < 6 else aw_recv1[h - 6])
                nc.gpsimd.collective_compute(
                    "AllReduce", ALU.add,
                    replica_groups=[list(range(NCORES))],
                    ins=[aw_send[6:12, :, :]], outs=[aw_recv1[:, :, :]])

            rv = load_w(p1, wd["rv_w"], E, "rv", "w1", 12)
            vt = project_tm(h1, rv, p1, "v", "v1")
            ro = load_w(p1, wd["ro_w"], E, "ro", "w1", 12)

            # ---- softmax (no max-subtraction) + transpose + context ----
            ctx = [p1.tile([128, T], BF16, name=f"ctx{m}", tag="ctx",
                           bufs=EC) for m in range(EC)]
            with tc.tile_pool(name="smps", bufs=1, space="PSUM") as smps, \
                 tc.tile_pool(name="cxps", bufs=1, space="PSUM") as cxps:
                for hc in range(EC):
                    pT = {}
                    for hb in range(2):
                        h = hc * 2 + hb
                        psb = []
                        for ic in range(2):
                            a = p1.tile([128, 256], BF16, name="awl",
                                        tag="awsb", bufs=4)
                            nc.sync.dma_start(
                                out=a,
                                in_=aw_src(h)[ic * 128:(ic + 1) * 128, :])
                            ex = p1.tile([128, 256], BF16, name="ex",
                                         tag="ex", bufs=4)
                            nc.scalar.activation(ex, a, AF.Exp, scale=S1)
                            sm = p1.tile([128, 1], F32, name="sm", tag="mx",
                                         bufs=8)
                            nc.vector.tensor_reduce(
                                sm, ex, axis=mybir.AxisListType.X, op=ALU.add)
                            rs = p1.tile([128, 1], F32, name="rs", tag="mxs",
                                         bufs=8)
                            nc.vector.reciprocal(rs, sm)
                            p = p1.tile([128, 256], BF16, name="p", tag="ex",
                                        bufs=4)
                            nc.vector.tensor_scalar_mul(p, ex, rs)
                            psb.append(p)
                        for jc in range(2):
                            tp = smps.tile([128, 256], BF16, name="tp",
                                           tag="tp", bufs=2)
                            for ic in range(2):
                                nc.tensor.transpose(
                                    tp[:, ic * 128:(ic + 1) * 128],
                                    psb[ic][:, jc * 128:(jc + 1) * 128],
                                    ident)
                            pt = p1.tile([128, 256], BF16,
                                         name=f"pT{h}_{jc}", tag="pt",
                                         bufs=8)
                            nc.vector.tensor_copy(out=pt, in_=tp)
                            pT[(hb, jc)] = pt
                    for r in range(RL):
                        ps = cxps.tile([128, 256], F32, name="cxp",
                                       tag="cx", bufs=4)
                        for hb in range(2):
                            h = hc * 2 + hb
                            for jc in range(2):
                                nc.tensor.matmul(
                                    ps[hb * 64:(hb + 1) * 64, :],
                                    vt[r * 2 + jc][:, h * 64:(h + 1) * 64],
                                    pT[(hb, jc)],
                                    start=(jc == 0), stop=(jc == 1))
                        nc.scalar.activation(
                            ctx[hc][:, r * 256:(r + 1) * 256], ps,
                            AF.Identity, bias=bias["rv_b"][hc])

            # ---- output proj + residual -> reshuffle -> a2a_send ----
            with tc.tile_pool(name="ops", bufs=1, space="PSUM") as ops:
                for m in range(EC):
                    x1 = p1.tile([128, T], F32, name=f"x1_{m}", tag="x1",
                                 bufs=2)
                    for s in range(NS):
                        ps = ops.tile([128, 512], F32, name="op", tag="op",
                                      bufs=4)
                        for k in range(EC):
                            nc.tensor.matmul(
                                ps, ro[k][:, m * 128:(m + 1) * 128],
                                ctx[k][:, s * 512:(s + 1) * 512],
                                start=(k == 0), stop=(k == EC - 1))
                        xr = p1.tile([128, 512], F32, name="xres",
                                     tag="xres", bufs=4)
                        nc.sync.dma_start(
                            out=xr, in_=x_in[m * 128:(m + 1) * 128,
                                             s * 512:(s + 1) * 512])
                        nc.vector.scalar_tensor_tensor(
                            out=x1[:, s * 512:(s + 1) * 512], in0=ps,
                            scalar=bias["ro_b"][m], in1=xr,
                            op0=ALU.add, op1=ALU.add)
                    # reshuffle t = r*256 + dest*32 + c -> (dest, c, r)
                    x1r = p1.tile([128, T], BF16, name=f"x1r{m}", tag="x1r",
                                  bufs=1)
                    in_ap = bass.AP(
                        tensor=x1.tensor, offset=x1.offset,
                        ap=[list(x1.ap[0]), [CL, NCORES], [1, CL], [256, RL]])
                    out_ap = bass.AP(
                        tensor=x1r.tensor, offset=x1r.offset,
                        ap=[list(x1r.ap[0]), [CL * RL, NCORES], [RL, CL],
                            [1, RL]])
                    nc.vector.tensor_copy(out=out_ap, in_=in_ap)
                    half, me = divmod(m, 3)
                    for dest in range(NCORES):
                        nc.sync.dma_start(
                            out=a2a_send[half, dest,
                                         me * 128:(me + 1) * 128, :],
                            in_=x1r[:, dest * 256:(dest + 1) * 256])
                    if m == 2 and stage == 3:
                        nc.gpsimd.collective_compute(
                            "AllToAll", ALU.bypass,
                            replica_groups=[list(range(NCORES))],
                            ins=[a2a_send[0]], outs=[a2a_recv[0]])

        if stage in (0, 1):
            with tc.tile_pool(name="st1", bufs=1) as st1:
                for m in range(EC):
                    t = st1.tile([128, T], F32, name="st1t", tag="t", bufs=2)
                    nc.vector.memset(t, 0.0)
                    nc.sync.dma_start(
                        out=y_out[m * 128:(m + 1) * 128, :], in_=t)
            return
        nc.gpsimd.collective_compute(
            "AllToAll", ALU.bypass, replica_groups=[list(range(NCORES))],
            ins=[a2a_send[1]], outs=[a2a_recv[1]])

        # ================= PHASE 2 (col shard) =================
        with tc.tile_pool(name="p2a", bufs=1) as p2a:
            # gather: recv[src][e][c*8+r] -> x2[e][c*64 + src*8 + r]
            x2 = [p2a.tile([128, T], BF16, name=f"x2_{m}", tag="x2", bufs=EC)
                  for m in range(EC)]
            for m in range(EC):
                for src in range(NCORES):
                    g = p2a.tile([128, CL * RL], BF16, name="x2g", tag="x2g",
                                 bufs=8)
                    half, me = divmod(m, 3)
                    nc.sync.dma_start(
                        out=g, in_=a2a_recv[half, src,
                                            me * 128:(me + 1) * 128, :])
                    out_ap = bass.AP(
                        tensor=x2[m].tensor, offset=x2[m].offset + src * RL,
                        ap=[list(x2[m].ap[0]), [R, CL], [1, RL]])
                    in_ap = bass.AP(
                        tensor=g.tensor, offset=g.offset,
                        ap=[list(g.ap[0]), [RL, CL], [1, RL]])
                    nc.vector.tensor_copy(out=out_ap, in_=in_ap)

            h2 = layernorm(lambda k, s: x2[k][:, s * 512:(s + 1) * 512],
                           p2a, "2", xbf16=True)
            cq = load_w(p2a, wd["cq_w"], E, "cq", "w2", 18)
            ck = load_w(p2a, wd["ck_w"], E, "ck", "w2", 18)
            cv = load_w(p2a, wd["cv_w"], E, "cv", "w2", 18)
            vt2 = project_tm(h2, cv, p2a, "v2", "v2")
            co = load_w(p2a, wd["co_w"], E, "co", "w2", 18)

            # pre-zeroed block-diagonal p^T tiles
            pbs = [p2a.tile([128, 128], BF16, name=f"pb{i}", tag="pbz",
                            bufs=8) for i in range(8)]
            for t in pbs:
                nc.vector.memset(t, 0.0)

            ctx2 = [p2a.tile([128, T], BF16, name=f"c2{m}", tag="ctx2",
                             bufs=EC) for m in range(EC)]
            with tc.tile_pool(name="qps2", bufs=1, space="PSUM") as qps2, \
                 tc.tile_pool(name="c2ps", bufs=1, space="PSUM") as c2ps:
                pbi = 0
                for hc in range(EC):
                    qt2 = project_chunk(h2, cq, bias["cq_b"][hc], hc, p2a,
                                        "q2", "q2", qps2)
                    kt2 = project_chunk(h2, ck, bias["ck_b"][hc], hc, p2a,
                                        "k2", "k2", qps2)
                    for g8 in range(4):           # 8 columns per group
                        c0 = g8 * 8
                        aw_ps = c2ps.tile([128, 512], F32, name="awp",
                                          tag="awp", bufs=2)
                        for ci in range(8):
                            c = c0 + ci
                            for hb in range(2):
                                nc.tensor.matmul(
                                    aw_ps[hb * 64:(hb + 1) * 64,
                                          ci * 64:(ci + 1) * 64],
                                    qt2[hb * 64:(hb + 1) * 64,
                                        c * 64:(c + 1) * 64],
                                    kt2[hb * 64:(hb + 1) * 64,
                                        c * 64:(c + 1) * 64],
                                    start=True, stop=True)
                        ex = p2a.tile([128, 8, 64], BF16, name="ex2",
                                      tag="csc", bufs=6)
                        nc.scalar.activation(
                            ex, aw_ps.rearrange("p (c j) -> p c j", j=64),
                            AF.Exp, scale=S2)
                        sm = p2a.tile([128, 8], F32, name="sm2", tag="mx2",
                                      bufs=4)
                        nc.vector.tensor_reduce(
                            sm, ex, axis=mybir.AxisListType.X, op=ALU.add)
                        rs = p2a.tile([128, 8], F32, name="rs2", tag="mxs2",
                                      bufs=4)
                        nc.vector.reciprocal(rs, sm)
                        rsb = bass.AP(tensor=rs.tensor, offset=rs.offset,
                                      ap=[list(rs.ap[0]), list(rs.ap[1]),
                                          [0, 64]])
                        p2 = p2a.tile([128, 8, 64], BF16, name="p2",
                                      tag="csc", bufs=6)
                        nc.vector.tensor_mul(p2, ex, rsb)
                        p2f_ = p2.rearrange("p c j -> p (c j)")
                        for b in range(4):        # col pairs within group
                            tp = c2ps.tile([128, 128], BF16, name="tp2",
                                           tag="tp2", bufs=1)
                            nc.tensor.transpose(
                                tp, p2f_[:, b * 128:(b + 1) * 128], ident)
                            cx = c2ps.tile([128, 128], F32, name="cx2",
                                           tag="cxp", bufs=2)
                            for hb in range(2):
                                pb = pbs[pbi % 8]
                                pbi += 1
                                nc.vector.tensor_copy(
                                    out=pb[0:64, 0:64],
                                    in_=tp[0:64, hb * 64:hb * 64 + 64])
                                nc.vector.tensor_copy(
                                    out=pb[64:128, 64:128],
                                    in_=tp[64:128, hb * 64:hb * 64 + 64])
                                tc_ = g8 * 4 + b
                                h = hc * 2 + hb
                                nc.tensor.matmul(
                                    cx[hb * 64:(hb + 1) * 64, :],
                                    vt2[tc_][:, h * 64:(h + 1) * 64],
                                    pb, start=True, stop=True)
                            nc.scalar.activation(
                                ctx2[hc][:, tc_ * 128:(tc_ + 1) * 128],
                                cx, AF.Identity, bias=bias["cv_b"][hc])

            # ---- output proj + residual -> x2p (SBUF, fp32) ----
            with tc.tile_pool(name="o2ps", bufs=1, space="PSUM") as o2ps:
                for m in range(EC):
                    for s in range(NS):
                        sl = slice(s * 512, (s + 1) * 512)
                        ps = o2ps.tile([128, 512], F32, name="o2p", tag="o2",
                                       bufs=4)
                        for k in range(EC):
                            nc.tensor.matmul(
                                ps, co[k][:, m * 128:(m + 1) * 128],
                                ctx2[k][:, sl],
                                start=(k == 0), stop=(k == EC - 1))
                        nc.vector.scalar_tensor_tensor(
                            out=x2p[m][:, sl], in0=ps,
                            scalar=bias["co_b"][m], in1=x2[m][:, sl],
                            op0=ALU.add, op1=ALU.add)

        # ---- FFN (own pool; x2p lives in gp) ----
        with tc.tile_pool(name="p2b", bufs=1) as p2b:
            h3 = layernorm(lambda k, s: x2p[k][:, s * 512:(s + 1) * 512],
                           p2b, "3", xbf16=True)
            f1 = load_w(p2b, wd["f1_w"], E, "f1", "f1w", EC)
            f2 = load_w(p2b, wd["f2_w"], F, "f2", "f2w", FC)
            with tc.tile_pool(name="fps", bufs=1, space="PSUM") as fps:
                for s in range(NS):
                    sl = slice(s * 512, (s + 1) * 512)
                    gm = []
                    for fc in range(FC):
                        ps = fps.tile([128, 512], F32, name="f1p", tag="f1p",
                                      bufs=3)
                        for k in range(EC):
                            nc.tensor.matmul(
                                ps, f1[k][:, fc * 128:(fc + 1) * 128],
                                h3[k][:, sl],
                                start=(k == 0), stop=(k == EC - 1))
                        g = p2b.tile([128, 512], BF16, name=f"gm{fc}",
                                     tag="gmid", bufs=FC + 2)
                        nc.scalar.activation(g, ps, AF.Gelu_apprx_tanh,
                                             bias=bias["f1_b"][fc])
                        gm.append(g)
                    for m in range(EC):
                        ps = fps.tile([128, 512], F32, name="f2p", tag="f2p",
                                      bufs=3)
                        for fc in range(FC):
                            nc.tensor.matmul(
                                ps, f2[fc][:, m * 128:(m + 1) * 128],
                                gm[fc], start=(fc == 0), stop=(fc == FC - 1))
                        ysl = p2b.tile([128, 512], F32, name="ysl",
                                       tag="ysl", bufs=4)
                        nc.vector.scalar_tensor_tensor(
                            out=ysl, in0=ps, scalar=bias["f2_b"][m],
                            in1=x2p[m][:, sl], op0=ALU.add, op1=ALU.add)
                        nc.sync.dma_start(
                            out=y_out[m * 128:(m + 1) * 128, sl], in_=ysl)


def _get_program(debug=False):
    key = ("prog", debug)
    if key not in _CACHE:
        _CACHE[key] = build_program(debug=debug)
    return _CACHE[key]


def make_in_maps(inputs, debug=False):
    bf = ml_dtypes.bfloat16
    f32 = np.float32
    x = np.asarray(inputs["x"], f32)                 # (64,256,1,768)
    g = {n: np.asarray(inputs[n], f32) for n in inputs if n != "num_heads"}
    # fold LN scale/bias into the following projections
    wcast, bkeep = {}, {}

    def fold(wn, bn, sn, lb):
        w = g[wn] * g[sn][:, None]
        b = g[bn] + g[lb] @ g[wn]
        return w, b

    for wn, bn in [("rq_w", "rq_b"), ("rk_w", "rk_b"), ("rv_w", "rv_b")]:
        w, b = fold(wn, bn, "ln1_s", "ln1_b")
        wcast[wn] = w
        bkeep[bn] = b
    for wn, bn in [("cq_w", "cq_b"), ("ck_w", "ck_b"), ("cv_w", "cv_b")]:
        w, b = fold(wn, bn, "ln2_s", "ln2_b")
        wcast[wn] = w
        bkeep[bn] = b
    w, b = fold("f1_w", "f1_b", "ln3_s", "ln3_b")
    wcast["f1_w"] = w
    bkeep["f1_b"] = b
    for wn in ["ro_w", "co_w", "f2_w"]:
        wcast[wn] = g[wn]
    for bn in ["ro_b", "co_b", "f2_b"]:
        bkeep[bn] = g[bn]

    wcast = {n: np.ascontiguousarray(w.astype(bf)) for n, w in wcast.items()}
    bkeep = {n: np.ascontiguousarray(b.astype(f32)) for n, b in bkeep.items()}
    # passthrough (unused by the program; keeps alternate harnesses happy)
    for n in ["ln1_s", "ln1_b", "ln2_s", "ln2_b", "ln3_s", "ln3_b"]:
        bkeep[n] = np.ascontiguousarray(g[n])
    in_maps = []
    for core in range(NCORES):
        xs = x[core * RL:(core + 1) * RL, :, 0, :].reshape(T, E)
        x_fm = np.ascontiguousarray(xs.T)            # (768, 2048)
        m = {"x_fm": x_fm}
        m.update(wcast)
        m.update(bkeep)
        in_maps.append(m)
    return in_maps


def gather_output(results):
    out = np.empty((R, C, 1, E), np.float32)
    for core in range(NCORES):
        y = results[core]["y"]                       # (768, 2048)
        # t' = i*64 + rg ;  y[e, i*64+rg] -> out[rg, core*32+i, 0, e]
        blk = y.reshape(E, CL, R).transpose(2, 1, 0)  # (64, 32, 768)
        out[:, core * CL:(core + 1) * CL, 0, :] = blk
    return out


def _jit_executable(nc):
    """Build (once per program) a persistent jitted executable so repeated
    kernel() calls reuse the loaded NEFF."""
    import jax
    from jax.sharding import Mesh, PartitionSpec
    from jax.experimental.shard_map import shard_map
    from concourse.bass2jax import (_bass_exec_p, install_neuronx_cc_hook,
                                    partition_id_tensor)

    install_neuronx_cc_hook()
    partition_name = (nc.partition_id_tensor.name
                      if nc.partition_id_tensor else None)
    in_names, out_names, out_avals, zero_shapes = [], [], [], []
    for alloc in nc.m.functions[0].allocations:
        if not isinstance(alloc, mybir.MemoryLocationSet):
            continue
        name = alloc.memorylocations[0].name
        if alloc.kind == "ExternalInput":
            if name != partition_name:
                in_names.append(name)
        elif alloc.kind == "ExternalOutput":
            out_names.append(name)
            shape = tuple(alloc.tensor_shape)
            dt = mybir.dt.np(alloc.dtype)
            out_avals.append(jax.core.ShapedArray(shape, dt))
            zero_shapes.append((shape, dt))
    all_in = in_names + out_names + ([partition_name] if partition_name
                                     else [])

    def _body(*args):
        operands = list(args)
        if partition_name is not None:
            operands.append(partition_id_tensor())
        return tuple(_bass_exec_p.bind(
            *operands, out_avals=tuple(out_avals), in_names=tuple(all_in),
            out_names=tuple(out_names), lowering_input_output_aliases=(),
            sim_require_finite=True, sim_require_nnan=True, nc=nc))

    devices = jax.devices()[:NCORES]
    mesh = Mesh(np.asarray(devices), ("core",))
    n_args = len(in_names) + len(out_names)
    jf = jax.jit(
        shard_map(_body, mesh=mesh,
                  in_specs=(PartitionSpec("core"),) * n_args,
                  out_specs=(PartitionSpec("core"),) * len(out_names),
                  check_rep=False),
        keep_unused=True)
    return jf, in_names, out_names, out_avals, zero_shapes


def run_cached(nc, in_maps):
    """Execute via the persistent jit executable; returns per-core dicts."""
    key = ("jit", id(nc))
    if key not in _CACHE:
        _CACHE[key] = _jit_executable(nc)
    jf, in_names, out_names, out_avals, zero_shapes = _CACHE[key]
    concat_in = [np.concatenate([np.asarray(m[n]) for m in in_maps], axis=0)
                 for n in in_names]
    concat_zero = [np.zeros((NCORES * s[0], *s[1:]), d)
                   for s, d in zero_shapes]
    outs = jf(*concat_in, *concat_zero)
    return [
        {n: np.asarray(outs[i]).reshape(NCORES, *out_avals[i].shape)[c]
         for i, n in enumerate(out_names)}
        for c in range(NCORES)
    ]


def kernel(**inputs):
    nc = _get_program(debug=False)
    in_maps = make_in_maps(inputs)
    if os.environ.get("K_NO_JIT_CACHE", ""):
        res = run_bass_kernel_spmd(nc, in_maps, list(range(NCORES)))
        return gather_output(res.results)
    return gather_output(run_cached(nc, in_maps))
